# revision 39
# baseline (speedup 1.0000x reference)
"""Trainium2 Bass kernel for nn_ModalityAdaptiveModule.

Reference computation (B=2, S=4096, D=512):
    tn = LN(text, g_t, b_t); im = LN(img, g_i, b_i)
    norms = concat([tn, im])                  # [4, S, D]
    K = concat([tn@Wkt.T+bkt, im@Wki.T+bki])  # [4, S, D]
    V = concat([tn@Wvt.T+bvt, im@Wvi.T+bvi])
    q = norms@Wq.T + bq
    attn = softmax(q@K.T / sqrt(D)); x = attn@V; x = x@Wo.T + bo
    out = concat([LN(x, g_t, b_t), LN(x, g_i, b_i)])  # [8, S, D]

Sharding: 8 cores = (attention batch b in 0..3) x (query half h in 0..1).
Each core gets its batch's full [S, D] input with its own query half
permuted to the front (key order is irrelevant to attention), computes
K/V over all S, Q over its 2048 rows, and writes both final-LN outputs
for its rows.

Device pipeline per core (transposed-operand formulation; all matmuls
f32r except probs@V in bf16):
  A = (x - mu) * rsig (LN affine folded into weights on host; rsig via
      ACT ln+exp so ACT never leaves the exp/ln table set)
  A_T via PE transpose; Q_T/K_T (transposed out), V (natural out)
  scores_T[tk, tq] = K_T.T @ Q_T  -> exp (no max-sub; scores max ~8.5)
  x_unnorm_T[d, tq] = sum_tk V-tiles.T @ U_T, plus ones-matmul -> sums
  y[tq, d] = x_T.T @ Wo.T * (1/sums) + bo   (diag-extract recip sums)
  out_m = ((y - mu2) * rsig2) * g_m + b_m for m in {t, i}
Loops are software-pipelined (skewed) so exp/evacuations overlap PE.
"""

import numpy as np
import ml_dtypes

import concourse.bass as bass
import concourse.mybir as mybir
import concourse.tile as tile
from concourse import bacc
from concourse.bass_utils import run_bass_kernel_spmd

AF = mybir.ActivationFunctionType
OP = mybir.AluOpType

# Pin ALL activations to the one table set that contains every function this
# kernel uses (exp, ln, copy, identity). The default chooser maps exp and ln
# to different sets, inserting a ~1.3us LoadActFuncSet per alternation
# (~120us/kernel). Emptying the other sets (order/ids preserved) forces a
# single load.
import concourse.hw_specs as _hw_specs
import functools as _functools

_ORIG_GET_ACT_TABLES = _hw_specs.get_activation_tables


@_functools.cache
def _pinned_act_tables(module_arch):
    full = _ORIG_GET_ACT_TABLES(module_arch)
    keep = "natural_log_exp_and_others"
    return {name: (funcs if name == keep else set())
            for name, funcs in full.items()}


ENABLE_ACT_PIN = True
if ENABLE_ACT_PIN:
    _hw_specs.get_activation_tables = _pinned_act_tables
    bacc.get_activation_tables = _pinned_act_tables
F32 = mybir.dt.float32
F32R = mybir.dt.float32r
BF16 = mybir.dt.bfloat16

D = 512
S = 4096          # keys per batch
TQ = 2048         # queries per core
DT = 4            # d tiles of 128
NKT = S // 128    # 32 key tiles
TC = 256          # phase-1 token chunk
NCH = S // TC     # 16 chunks
TQB = 512         # tq block
NBLK = TQ // TQB  # 4 blocks
EPS = 1e-5


def build_kernel():
    nc = bacc.Bacc("TRN2", target_bir_lowering=False, debug=False,
                   enable_asserts=True, num_devices=8)

    x_d = nc.dram_tensor("x", [S, D], F32, kind="ExternalInput").ap()
    gqt_d = nc.dram_tensor("gqt", [D, D], F32R, kind="ExternalInput").ap()
    gkt_d = nc.dram_tensor("gkt", [D, D], F32R, kind="ExternalInput").ap()
    gvt_d = nc.dram_tensor("gvt", [D, D], F32R, kind="ExternalInput").ap()
    wot_d = nc.dram_tensor("wot", [D, D], F32R, kind="ExternalInput").ap()
    cq_d = nc.dram_tensor("cq", [D], F32, kind="ExternalInput").ap()
    ck_d = nc.dram_tensor("ck", [D], F32, kind="ExternalInput").ap()
    cv_d = nc.dram_tensor("cv", [D], F32, kind="ExternalInput").ap()
    bo_d = nc.dram_tensor("bo", [D], F32, kind="ExternalInput").ap()
    g2t_d = nc.dram_tensor("g2t", [D], F32, kind="ExternalInput").ap()
    b2t_d = nc.dram_tensor("b2t", [D], F32, kind="ExternalInput").ap()
    g2i_d = nc.dram_tensor("g2i", [D], F32, kind="ExternalInput").ap()
    b2i_d = nc.dram_tensor("b2i", [D], F32, kind="ExternalInput").ap()
    ident_d = nc.dram_tensor("ident", [128, 128], F32, kind="ExternalInput").ap()
    onesr_d = nc.dram_tensor("onesr", [128, 128], F32R, kind="ExternalInput").ap()
    out_d = nc.dram_tensor("out2", [2, TQ, D], F32, kind="ExternalOutput").ap()

    def bcast(vec_ap, parts=128):
        return bass.AP(tensor=vec_ap.tensor, offset=vec_ap.offset,
                       ap=[[0, parts]] + list(vec_ap.ap))

    with tile.TileContext(nc) as tc:
        with (
            tc.tile_pool(name="persist", bufs=1) as persist,
            tc.tile_pool(name="resident", bufs=1) as resident,
        ):
            # ---- critical-path first: identity + first x chunk DMA ----
            ident = persist.tile([128, 128], F32)
            nc.sync.dma_start(ident[:], ident_d)
            xc0 = persist.tile([128, 2, D], F32)
            nc.sync.dma_start(
                xc0[:], x_d[0:TC, :].rearrange("(s p) d -> p s d", p=128))
            eps_t = persist.tile([128, 1], F32)
            nc.vector.memset(eps_t[:], EPS)
            ones_bf = persist.tile([128, 128], BF16)
            nc.vector.memset(ones_bf[:], 1.0)
            ones_r = persist.tile([128, 128], F32R)
            nc.sync.dma_start(ones_r[:], onesr_d)
            cq_s = persist.tile([128, DT], F32)
            nc.sync.dma_start(cq_s[:], cq_d.rearrange("(o p) -> p o", p=128))
            ck_s = persist.tile([128, DT], F32)
            nc.sync.dma_start(ck_s[:], ck_d.rearrange("(o p) -> p o", p=128))
            cv_rep = persist.tile([128, D], F32)
            nc.gpsimd.dma_start(cv_rep[:], bcast(cv_d))

            # ---- resident big tensors ----
            KT = resident.tile([128, DT, S], F32R)
            QT = resident.tile([128, DT, TQ], F32R)
            Vb = resident.tile([128, NKT, D], BF16)
            wot_s = resident.tile([128, DT, D], F32R)

            def rsig_lnexp(pool, var_ap, tag):
                """1/sqrt(var+eps) via exp(-0.5*ln(var+eps)) — stays in the
                exp/ln ACT table set (no LoadActFuncSet thrash)."""
                lnv = pool.tile([128, 1], F32, tag=f"lnv{tag}", name=f"lnv{tag}")
                nc.scalar.activation(lnv[:], var_ap, AF.Ln, bias=eps_t[:, 0:1],
                                     scale=1.0)
                rs = pool.tile([128, 1], F32, tag=f"rsx{tag}", name=f"rsx{tag}")
                nc.scalar.activation(rs[:], lnv[:], AF.Exp, scale=-0.5)
                return rs

            # ================= PHASE 1: LN + transpose + QKV =================
            with (
                tc.tile_pool(name="p1w", bufs=1) as p1w,
                tc.tile_pool(name="p1x", bufs=2) as p1x,
                tc.tile_pool(name="p1s", bufs=3) as p1s,
                tc.tile_pool(name="p1ps", bufs=2, space="PSUM") as p1ps,
                tc.tile_pool(name="p1pk", bufs=3, space="PSUM") as p1pk,
                tc.tile_pool(name="p1pv", bufs=2, space="PSUM") as p1pv,
            ):
                gkt_s = p1w.tile([128, DT, D], F32R)
                nc.sync.dma_start(gkt_s[:], gkt_d.rearrange("(i p) o -> p i o", p=128))
                gvt_s = p1w.tile([128, DT, D], F32R)
                nc.sync.dma_start(gvt_s[:], gvt_d.rearrange("(i p) o -> p i o", p=128))
                gqt_s = p1w.tile([128, DT, D], F32R)
                nc.sync.dma_start(gqt_s[:], gqt_d.rearrange("(i p) o -> p i o", p=128))

                def ln_transpose(c):
                    if c == 0:
                        xc = xc0
                    else:
                        xc = p1x.tile([128, 2, D], F32, tag="xc", name=f"xc{c}")
                        nc.sync.dma_start(
                            xc[:], x_d[c * TC:(c + 1) * TC, :].rearrange(
                                "(s p) d -> p s d", p=128))
                    Ac = p1x.tile([128, 2, D], F32, tag="ac", name=f"ac{c}", bufs=3)
                    AcT = p1x.tile([128, DT, TC], F32R, tag="act", name=f"act{c}", bufs=3)
                    for s in range(2):
                        stats = p1s.tile([128, 6], F32, tag="st", name=f"st{c}_{s}")
                        nc.vector.bn_stats(stats[:], xc[:, s, :])
                        mv = p1s.tile([128, 2], F32, tag="mv", name=f"mv{c}_{s}")
                        nc.vector.bn_aggr(mv[:], stats[:])
                        rs = rsig_lnexp(p1s, mv[:, 1:2], "1")
                        nmr = p1s.tile([128, 1], F32, tag="nmr", name=f"nmr{c}_{s}")
                        nc.vector.tensor_scalar(
                            out=nmr[:], in0=mv[:, 0:1], scalar1=rs[:, 0:1],
                            scalar2=-1.0, op0=OP.mult, op1=OP.mult)
                        nc.scalar.activation(Ac[:, s, :], xc[:, s, :], AF.Identity,
                                             bias=nmr[:, 0:1], scale=rs[:, 0:1])
                        for dt in range(DT):
                            tp = p1ps.tile([128, 128], F32, tag="tp",
                                           name=f"tp{c}_{s}_{dt}")
                            nc.tensor.transpose(
                                tp[:], Ac[:, s, dt * 128:(dt + 1) * 128], ident[:])
                            nc.vector.tensor_copy(
                                AcT[:, dt, s * 128:(s + 1) * 128], tp[:])
                    return AcT

                def projections(c, AcT):
                    for o in range(DT):
                        pk = p1pk.tile([128, TC], F32, tag="pk", name=f"pk{c}_{o}")
                        for i in range(DT):
                            nc.tensor.matmul(
                                pk[:], gkt_s[:, i, o * 128:(o + 1) * 128],
                                AcT[:, i, :], start=(i == 0), stop=(i == DT - 1))
                        nc.scalar.activation(KT[:, o, c * TC:(c + 1) * TC], pk[:],
                                             AF.Identity, bias=ck_s[:, o:o + 1],
                                             scale=1.0)
                    for s in range(2):
                        pv = p1pv.tile([128, D], F32, tag="pv", name=f"pv{c}_{s}")
                        for i in range(DT):
                            nc.tensor.matmul(
                                pv[:], AcT[:, i, s * 128:(s + 1) * 128],
                                gvt_s[:, i, :], start=(i == 0), stop=(i == DT - 1))
                        nc.vector.tensor_add(Vb[:, c * 2 + s, :], pv[:], cv_rep[:])
                    if c < NCH // 2:
                        for o in range(DT):
                            pq = p1pk.tile([128, TC], F32, tag="pk",
                                           name=f"pq{c}_{o}")
                            for i in range(DT):
                                nc.tensor.matmul(
                                    pq[:], gqt_s[:, i, o * 128:(o + 1) * 128],
                                    AcT[:, i, :], start=(i == 0), stop=(i == DT - 1))
                            nc.scalar.activation(QT[:, o, c * TC:(c + 1) * TC],
                                                 pq[:], AF.Identity,
                                                 bias=cq_s[:, o:o + 1], scale=1.0)

                # skewed: transpose chunk c while projecting chunk c-1
                prev = ln_transpose(0)
                for c in range(1, NCH):
                    cur = ln_transpose(c)
                    projections(c - 1, prev)
                    prev = cur
                projections(NCH - 1, prev)

            nc.sync.dma_start(wot_s[:], wot_d.rearrange("(i p) o -> p i o", p=128))

            # ============ PHASE 2/3: attention + out-proj + final LN ============
            with (
                tc.tile_pool(name="p2u", bufs=8) as p2u,
                tc.tile_pool(name="p2s", bufs=2) as p2s,
                tc.tile_pool(name="p2y", bufs=2) as p2y,
                tc.tile_pool(name="p2o", bufs=2) as p2o,
                tc.tile_pool(name="p2st", bufs=3) as p2st,
                tc.tile_pool(name="p2sum", bufs=2) as p2sum,
                tc.tile_pool(name="p2c", bufs=1) as p2c,
                tc.tile_pool(name="psc", bufs=3, space="PSUM") as psc,
                tc.tile_pool(name="pxv", bufs=1, space="PSUM") as pxv,
                tc.tile_pool(name="psum_y", bufs=1, space="PSUM") as psum_y,
            ):
                bo_rep = p2c.tile([128, D], F32)
                nc.gpsimd.dma_start(bo_rep[:], bcast(bo_d))
                g2t_rep = p2c.tile([128, D], F32)
                nc.gpsimd.dma_start(g2t_rep[:], bcast(g2t_d))
                b2t_rep = p2c.tile([128, D], F32)
                nc.gpsimd.dma_start(b2t_rep[:], bcast(b2t_d))
                g2i_rep = p2c.tile([128, D], F32)
                nc.gpsimd.dma_start(g2i_rep[:], bcast(g2i_d))
                b2i_rep = p2c.tile([128, D], F32)
                nc.gpsimd.dma_start(b2i_rep[:], bcast(b2i_d))

                def oproj_ln(q0, w, xT, rcp):
                    for j in range(w // 128):
                        py = psum_y.tile([128, D], F32, tag="py",
                                         name=f"py{q0}_{j}")
                        for dt in range(DT):
                            nc.tensor.matmul(
                                py[:], xT[:, dt, j * 128:(j + 1) * 128],
                                wot_s[:, dt, :], start=(dt == 0),
                                stop=(dt == DT - 1))
                        y = p2y.tile([128, D], F32, tag="y", name=f"y{q0}_{j}")
                        nc.vector.tensor_scalar_mul(y[:], py[:], rcp[:, j:j + 1])
                        nc.vector.tensor_add(y[:], y[:], bo_rep[:])
                        stats = p2st.tile([128, 6], F32, tag="st2",
                                          name=f"st2_{q0}_{j}")
                        nc.vector.bn_stats(stats[:], y[:])
                        mv = p2st.tile([128, 2], F32, tag="mv2",
                                       name=f"mv2_{q0}_{j}")
                        nc.vector.bn_aggr(mv[:], stats[:])
                        rs2 = rsig_lnexp(p2st, mv[:, 1:2], "2")
                        nmr2 = p2st.tile([128, 1], F32, tag="nmr2",
                                         name=f"nmr2_{q0}_{j}")
                        nc.vector.tensor_scalar(
                            out=nmr2[:], in0=mv[:, 0:1], scalar1=rs2[:, 0:1],
                            scalar2=-1.0, op0=OP.mult, op1=OP.mult)
                        n2 = p2y.tile([128, D], F32, tag="n2", name=f"n2_{q0}_{j}")
                        nc.scalar.activation(n2[:], y[:], AF.Identity,
                                             bias=nmr2[:, 0:1], scale=rs2[:, 0:1])
                        r0 = q0 + j * 128
                        for m, (g_rep, b_rep) in enumerate(
                                [(g2t_rep, b2t_rep), (g2i_rep, b2i_rep)]):
                            om = p2o.tile([128, D], F32, tag=f"om{m}",
                                          name=f"om{m}_{q0}_{j}")
                            nc.vector.tensor_mul(om[:], n2[:], g_rep[:])
                            nc.vector.tensor_add(om[:], om[:], b_rep[:])
                            nc.sync.dma_start(out_d[m, r0:r0 + 128, :], om[:])

                prev_oproj = None
                # taper: last 512 queries as two 256-wide blocks so the final
                # (unoverlapped) evac+O-proj+LN chain is half as long
                BLOCKS = [(0, 512), (512, 512), (1024, 512),
                          (1536, 256), (1792, 256)]
                for blk, (q0, w) in enumerate(BLOCKS):
                    pxs = [pxv.tile([128, w], F32, tag=f"px{dt}",
                                    name=f"px{dt}_{blk}") for dt in range(DT)]
                    psm = psc.tile([128, w], F32, tag="ps", name=f"psm{blk}")
                    # softmax sums accumulate on DVE (frees ~27us of PE);
                    # one ones-matmul per block replicates them across
                    # partitions for the diagonal extraction.
                    sacc = p2sum.tile([128, w], F32R, tag="sacc",
                                      name=f"sacc{blk}")
                    Us = [None] * NKT
                    # skewed: scores/exp for k, attnV for k-1; prev block's
                    # O-proj emitted mid-loop so it overlaps this block's attn
                    for k in range(NKT + 1):
                        if k == 12 and prev_oproj is not None:
                            oproj_ln(*prev_oproj)
                            prev_oproj = None
                        if k < NKT:
                            ps = psc.tile([128, w], F32, tag="ps",
                                          name=f"ps{blk}_{k}")
                            for i in range(DT):
                                nc.tensor.matmul(
                                    ps[:], KT[:, i, k * 128:(k + 1) * 128],
                                    QT[:, i, q0:q0 + w],
                                    start=(i == 0), stop=(i == DT - 1))
                            U = p2u.tile([128, w], BF16, tag="ut",
                                         name=f"ut{blk}_{k}")
                            nc.scalar.activation(U[:], ps[:], AF.Exp)
                            Us[k] = U
                        if k >= 1:
                            kk = k - 1
                            Ukk = Us[kk]
                            for dt in range(DT):
                                nc.tensor.matmul(
                                    pxs[dt][:], Vb[:, kk, dt * 128:(dt + 1) * 128],
                                    Ukk[:], start=(kk == 0), stop=(kk == NKT - 1))
                            if kk == 0:
                                nc.vector.tensor_copy(sacc[:], Ukk[:])
                            else:
                                nc.vector.tensor_add(sacc[:], sacc[:], Ukk[:])
                            Us[kk] = None
                    nc.tensor.matmul(psm[:], ones_r[:], sacc[:],
                                     start=True, stop=True)
                    # evacuate x_T and recip(sums)
                    xT = p2s.tile([128, DT, w], F32R, tag="xt", name=f"xt{blk}")
                    for dt in range(DT):
                        nc.scalar.copy(xT[:, dt, :], pxs[dt][:])
                    rcp = p2st.tile([128, w // 128], F32, tag="rcp",
                                    name=f"rcp{blk}")
                    for j in range(w // 128):
                        dg = p2st.tile([128, 128], F32, tag="dg",
                                       name=f"dg{blk}_{j}")
                        nc.vector.tensor_mul(dg[:], psm[:, j * 128:(j + 1) * 128],
                                             ident[:])
                        nc.vector.reduce_sum(out=rcp[:, j:j + 1], in_=dg[:],
                                             axis=mybir.AxisListType.X)
                    nc.vector.reciprocal_approx_fast(rcp[:], rcp[:])
                    prev_oproj = (q0, w, xT, rcp)
                oproj_ln(*prev_oproj)
    nc.compile()
    return nc


_NC_CACHE = None


def _get_nc():
    global _NC_CACHE
    if _NC_CACHE is None:
        _NC_CACHE = build_kernel()
    return _NC_CACHE


def _prep_core_inputs(text, img, ln_t_g, ln_t_b, ln_i_g, ln_i_b,
                      Wq, bq, Wkt, bkt, Wvt, bvt, Wki, bki, Wvi, bvi, Wo, bo):
    s = np.float32(D) ** -0.5
    ident = np.eye(128, dtype=np.float32)
    in_maps = []
    for core in range(8):
        b, h = core // 2, core % 2
        m_t = b < 2
        x = np.asarray(text[b] if m_t else img[b - 2], np.float32)
        if h == 1:
            x = np.concatenate([x[TQ:], x[:TQ]], axis=0)
        g = np.asarray(ln_t_g if m_t else ln_i_g, np.float32)
        bb = np.asarray(ln_t_b if m_t else ln_i_b, np.float32)
        Wk, bk = (Wkt, bkt) if m_t else (Wki, bki)
        Wv, bv = (Wvt, bvt) if m_t else (Wvi, bvi)
        Wq_, bq_, Wk, bk, Wv, bv, Wo_, bo_ = [
            np.asarray(a, np.float32) for a in (Wq, bq, Wk, bk, Wv, bv, Wo, bo)]
        in_maps.append({
            "x": np.ascontiguousarray(x),
            "gqt": np.ascontiguousarray((Wq_ * g[None, :]).T * s),
            "gkt": np.ascontiguousarray((Wk * g[None, :]).T),
            "gvt": np.ascontiguousarray((Wv * g[None, :]).T),
            "wot": np.ascontiguousarray(Wo_.T),
            "cq": np.ascontiguousarray((Wq_ @ bb + bq_) * s),
            "ck": np.ascontiguousarray(Wk @ bb + bk),
            "cv": np.ascontiguousarray(Wv @ bb + bv),
            "bo": np.ascontiguousarray(bo_),
            "g2t": np.ascontiguousarray(np.asarray(ln_t_g, np.float32)),
            "b2t": np.ascontiguousarray(np.asarray(ln_t_b, np.float32)),
            "g2i": np.ascontiguousarray(np.asarray(ln_i_g, np.float32)),
            "b2i": np.ascontiguousarray(np.asarray(ln_i_b, np.float32)),
            "ident": ident,
            "onesr": np.ones((128, 128), np.float32),
        })
    return in_maps


def kernel(**inputs):
    kr = kernel_raw(**inputs)
    return kr[0]


def kernel_raw(**inputs):
    """Returns (full_output, BassKernelResults)."""
    import time as _time
    nc = _get_nc()
    in_maps = _prep_core_inputs(**inputs)
    res = None
    last_exc = None
    for attempt in range(6):
        try:
            res = run_bass_kernel_spmd(nc, in_maps, core_ids=list(range(8)))
            break
        except Exception as e:  # transient device wedge self-heals in ~1-3 min
            last_exc = e
            if "UNAVAILABLE" not in str(e) and "INTERNAL" not in str(e):
                raise
            _time.sleep(30)
    if res is None:
        raise last_exc
    out = np.zeros((8, S, D), np.float32)
    for core in range(8):
        b, h = core // 2, core % 2
        o2 = res.results[core]["out2"]
        out[b, h * TQ:(h + 1) * TQ] = o2[0]
        out[4 + b, h * TQ:(h + 1) * TQ] = o2[1]
    return out, res
